# revision 3
# baseline (speedup 1.0000x reference)
"""Trainium2 Bass kernel for nn_AttentionModel_39848706572868.

Multi-head attention with fixed-key dropout:
  out = dropout(softmax(Q K^T / 8)) @ V     for B=2, S=2048, H=16, D=64.

Strategy (8 NeuronCores, head-parallel):
  - 32 (batch, head) pairs are sharded 4-per-core. No cross-core comms.
  - The dropout mask is a deterministic function of jax.random.key(42); it is
    reproduced bit-exactly on the host (CPU threefry) and shipped to the
    device as a bf16 {0,1} drop-mask.
  - On device, everything is computed in a "transposed scores" layout
    S^T[t, s] so that the P@V contraction consumes tiles in natural layout:
      mm1:   S^T[tj, si] = K^T[d, tj].T @ Q^T[d, si]            (PE, f32r)
      exp:   E = exp(S^T)                                       (ACT, ->bf16)
      mask:  Ed = E * dropmask^T                                (DVE, bf16 2x)
      mm2a:  acc[si, 0:64] += [V|1][tj,:].T @ E[tj, si]   (PE accum, bf16)
             -> unmasked sum(e*v) in rows 0..63, denominator sum(e) in row 64
      mm2b:  acc[si, 0:64] += [-V|0][tj,:].T @ Ed[tj, si] (PE accum, bf16)
             -> subtracts the dropped 10%: rows 0..63 become sum(m*e*v)
  - Host finishes: out = num / (0.9 * denom), transpose back to [B,H,S,D].

The kernel is self-contained: it hardcodes shapes and builds/caches the Bass
program on first call.
"""

import sys

for _p in ("/opt/trn_rl_repo",):
    if _p not in sys.path:
        sys.path.insert(0, _p)

import numpy as np
import ml_dtypes

import concourse.bacc as bacc
import concourse.tile as tile
from concourse import mybir
from concourse.bass_utils import run_bass_kernel_spmd

# Problem constants
B, S, H, D = 2, 2048, 16, 64
N_CORES = 8
PAIRS_TOTAL = B * H            # 32
PAIRS = PAIRS_TOTAL // N_CORES  # 4 per core
DROP_P = 0.1
KEEP_P = 1.0 - DROP_P

BF16 = mybir.dt.bfloat16
F32 = mybir.dt.float32
F32R = mybir.dt.float32r


def build_nc(pairs=PAIRS, s=S, d=D, iblk=1024, jt=128, score_dt=F32R):
    """Build the per-core Bass program (SPMD: same program, 8 cores)."""
    nj = s // jt          # number of key tiles
    nib = s // iblk       # number of query blocks
    mmw = min(iblk, 512)  # matmul moving-dim chunk (one psum bank of fp32)
    nh = iblk // mmw
    nc = bacc.Bacc("TRN2", target_bir_lowering=False, debug=False)

    # DRAM I/O (per-core shard). qt/kt are pre-scaled (q already divided by
    # inv_scale_factor on host) and pre-transposed to [pair, d, s].
    qt = nc.dram_tensor("qt", [pairs, d, s], score_dt, kind="ExternalInput").ap()
    kt = nc.dram_tensor("kt", [pairs, d, s], score_dt, kind="ExternalInput").ap()
    # va: [pair, 128, nj, d+1] bf16 — V tile columns plus a ones column.
    va = nc.dram_tensor("va", [pairs, jt, nj, d + 1], BF16, kind="ExternalInput").ap()
    # vb: same layout, holding -V and a zeros column.
    vb = nc.dram_tensor("vb", [pairs, jt, nj, d + 1], BF16, kind="ExternalInput").ap()
    # maskd: transposed drop-mask (1-keep) in bf16 {0,1}: [pair, t, s]
    maskd = nc.dram_tensor("maskd", [pairs, s, s], BF16, kind="ExternalInput").ap()
    # outc: rows 0..d-1 = numerator^T, row d = denominator
    outc = nc.dram_tensor("outc", [pairs, d + 1, s], F32, kind="ExternalOutput").ap()

    with tile.TileContext(nc) as tc:
        with (
            tc.tile_pool(name="qk", bufs=2) as qk_pool,
            tc.tile_pool(name="vw", bufs=2) as vw_pool,
            tc.tile_pool(name="expu", bufs=3) as eu_pool,
            tc.tile_pool(name="expd", bufs=3) as ed_pool,
            tc.tile_pool(name="mask", bufs=4) as m_pool,
            tc.tile_pool(name="outs", bufs=2) as o_pool,
            tc.tile_pool(name="ps_scores", bufs=3, space="PSUM") as ps_s,
            tc.tile_pool(name="ps_acc", bufs=1, space="PSUM") as ps_a,
        ):
            for p in range(pairs):
                qt_sb = qk_pool.tile([d, s], score_dt, tag="qt")
                kt_sb = qk_pool.tile([d, s], score_dt, tag="kt")
                nc.sync.dma_start(out=qt_sb, in_=qt[p])
                nc.sync.dma_start(out=kt_sb, in_=kt[p])
                va_sb = vw_pool.tile([jt, nj, d + 1], BF16, tag="va")
                vb_sb = vw_pool.tile([jt, nj, d + 1], BF16, tag="vb")
                nc.sync.dma_start(out=va_sb, in_=va[p])
                nc.sync.dma_start(out=vb_sb, in_=vb[p])

                for ib in range(nib):
                    i0 = ib * iblk
                    acc = ps_a.tile([d + 1, iblk], F32, tag="acc")
                    for j in range(nj):
                        # mm1: scores^T tile [jt, iblk] (f32 psum)
                        ps = ps_s.tile([jt, iblk], F32, tag="scores")
                        for h in range(nh):
                            nc.tensor.matmul(
                                ps[:, h * mmw:(h + 1) * mmw],
                                lhsT=kt_sb[:, j * jt:(j + 1) * jt],
                                rhs=qt_sb[:, i0 + h * mmw:i0 + (h + 1) * mmw],
                                start=True,
                                stop=True,
                            )
                        # exp (unmasked) -> bf16 SBUF
                        eu = eu_pool.tile([jt, iblk], BF16, tag="eu")
                        nc.scalar.activation(
                            out=eu, in_=ps,
                            func=mybir.ActivationFunctionType.Exp,
                        )
                        # dropped exp: ed = eu * dropmask
                        msk = m_pool.tile([jt, iblk], BF16, tag="msk")
                        nc.sync.dma_start(
                            out=msk,
                            in_=maskd[p, j * jt:(j + 1) * jt, i0:i0 + iblk],
                        )
                        ed = ed_pool.tile([jt, iblk], BF16, tag="ed")
                        nc.vector.tensor_mul(ed, eu, msk)
                        # mm2a/mm2b accumulate into acc
                        for h in range(nh):
                            hs = slice(h * mmw, (h + 1) * mmw)
                            nc.tensor.matmul(
                                acc[:, hs],
                                lhsT=va_sb[:, j, :],
                                rhs=eu[:, hs],
                                start=(j == 0),
                                stop=False,
                            )
                            nc.tensor.matmul(
                                acc[:, hs],
                                lhsT=vb_sb[:, j, :],
                                rhs=ed[:, hs],
                                start=False,
                                stop=(j == nj - 1),
                            )
                    # acc -> SBUF -> DRAM
                    out_sb = o_pool.tile([d + 1, iblk], F32, tag="osb")
                    nc.vector.tensor_copy(out_sb, acc)
                    nc.sync.dma_start(out=outc[p, :, i0:i0 + iblk], in_=out_sb)

    nc.compile()
    return nc


# ---------------------------------------------------------------------------
# Host-side data preparation

_MASK_CACHE = {}


def _get_drop_mask_T(b=B, h=H, s=S):
    """Bit-exact reproduction of the reference dropout mask, transposed.

    Returns drop-mask (1 - keep) as bf16 [b, h, s(t), s(q)]."""
    key_shape = (b, h, s, s)
    if key_shape in _MASK_CACHE:
        return _MASK_CACHE[key_shape]
    import jax

    cpu = jax.devices("cpu")[0]
    with jax.default_device(cpu):
        keep = jax.random.bernoulli(jax.random.key(42), KEEP_P, key_shape)
        keep = np.asarray(keep)
    dropT = (~keep.transpose(0, 1, 3, 2)).astype(ml_dtypes.bfloat16)
    _MASK_CACHE[key_shape] = dropT
    return dropT


_NC_CACHE = {}


def _get_nc():
    if "nc" not in _NC_CACHE:
        _NC_CACHE["nc"] = build_nc()
    return _NC_CACHE["nc"]


def _prepare_in_maps(query, key, value, inv_scale_factor):
    """Shard + lay out the full inputs for the 8 cores."""
    scale = 1.0 / np.float32(inv_scale_factor)
    # [B,S,H,D] -> [B,H,D,S] -> [32, D, S]
    qt = np.ascontiguousarray(
        (query * scale).transpose(0, 2, 3, 1).reshape(PAIRS_TOTAL, D, S),
        dtype=np.float32)
    kt = np.ascontiguousarray(
        key.transpose(0, 2, 3, 1).reshape(PAIRS_TOTAL, D, S), dtype=np.float32)
    # V: [B,S,H,D] -> [B,H,S,D] -> [32, S, D] -> tiles [32, 128, nj, D]
    v = value.transpose(0, 2, 1, 3).reshape(PAIRS_TOTAL, S, D)
    nj = S // 128
    vt = v.reshape(PAIRS_TOTAL, nj, 128, D).transpose(0, 2, 1, 3)  # [32,128,nj,D]
    va = np.zeros((PAIRS_TOTAL, 128, nj, D + 1), dtype=ml_dtypes.bfloat16)
    vb = np.zeros((PAIRS_TOTAL, 128, nj, D + 1), dtype=ml_dtypes.bfloat16)
    va[..., :D] = vt.astype(ml_dtypes.bfloat16)
    va[..., D] = 1.0
    vb[..., :D] = (-va[..., :D].astype(np.float32)).astype(ml_dtypes.bfloat16)

    dropT = _get_drop_mask_T().reshape(PAIRS_TOTAL, S, S)

    in_maps = []
    for c in range(N_CORES):
        sl = slice(c * PAIRS, (c + 1) * PAIRS)
        in_maps.append({
            "qt": qt[sl],
            "kt": kt[sl],
            "va": np.ascontiguousarray(va[sl]),
            "vb": np.ascontiguousarray(vb[sl]),
            "maskd": np.ascontiguousarray(dropT[sl]),
        })
    return in_maps


def _assemble_output(results):
    """results: list of out_maps per core -> full [B,H,S,D] fp32 output."""
    outc = np.concatenate([r["outc"] for r in results], axis=0)  # [32, D+1, S]
    num = outc[:, :D, :]                  # [32, D, S] = numerator^T
    den = outc[:, D, :]                   # [32, S]
    out_t = num / (KEEP_P * den[:, None, :])
    # [32, D, S] -> [32, S, D] -> [B, H, S, D]
    return np.ascontiguousarray(
        out_t.transpose(0, 2, 1).reshape(B, H, S, D).astype(np.float32))


def run(query, key, value, inv_scale_factor, trace=False, tmpdir=None):
    nc = _get_nc()
    in_maps = _prepare_in_maps(
        np.asarray(query, dtype=np.float32),
        np.asarray(key, dtype=np.float32),
        np.asarray(value, dtype=np.float32),
        np.float32(inv_scale_factor),
    )
    res = run_bass_kernel_spmd(
        nc, in_maps, core_ids=list(range(N_CORES)), trace=trace, tmpdir=tmpdir,
    )
    return _assemble_output(res.results), res


def kernel(query, key, value, inv_scale_factor):
    out, _ = run(query, key, value, inv_scale_factor)
    return out
